# revision 1
# baseline (speedup 1.0000x reference)
"""GAT-style message passing kernel for Trainium2 (8 NeuronCores, SPMD).

h_prime[i] = (sum_j exp(lrelu(<label_i,label_j>)) * h[j]) / rowsum_i @ W

Key algebraic transform: W commutes with the segment-sum, so we aggregate raw
h[dst] rows per src node and apply W once per node (100k rows) instead of per
edge (1.6M rows).  The softmax denominator is obtained from a constant ones
column appended to the gathered feature rows, so a single selector matmul per
128-edge tile computes both the weighted sum and the row-sum.

Sharding: nodes (src) are split 12500/core; each core gets all edges whose src
it owns, sorted by src.  One NEFF runs SPMD on all 8 cores; the per-block tile
schedule is the max across cores so the program is identical.
"""

import sys

sys.path.insert(0, "/opt/trn_rl_repo")

import json

import numpy as np

import concourse.bass as bass
import concourse.mybir as mybir
from concourse.bass import IndirectOffsetOnAxis
from concourse.bass_utils import run_bass_kernel_spmd

from concourse.tile import TileContext


def _legalize_waits(bir: bytes) -> bytes:
    """This toolchain's codegen allows one sync-wait per instruction; move
    extras onto injected wait-only EventSemaphore ops in the same queue."""
    d = json.loads(bir)
    n = 0
    for fn in d["functions"]:
        for blk in fn["blocks"]:
            out = []
            for inst in blk["instructions"]:
                si = inst.get("sync_info")
                ow = (si or {}).get("on_wait") or []
                if len(ow) > 1:
                    for w in ow[:-1]:
                        n += 1
                        out.append(
                            {
                                "debug": inst.get("debug", 0),
                                "engine": inst.get("engine"),
                                "ins": [],
                                "name": f"waitfix_{n}_{inst['name']}",
                                "opcode": "EventSemaphore",
                                "outs": [],
                                "sync_info": {"on_update": [], "on_wait": [w]},
                            }
                        )
                    si["on_wait"] = [ow[-1]]
                out.append(inst)
            blk["instructions"] = out
    return json.dumps(d).encode()


_orig_to_json_bytes = bass.Bass.to_json_bytes


def _patched_to_json_bytes(self):
    return _legalize_waits(_orig_to_json_bytes(self))


bass.Bass.to_json_bytes = _patched_to_json_bytes

N = 100000
E = 1600000
IN_F = 256
D_LABEL = 32
OUT_F = 256
ALPHA = 0.2
EPS = 1e-9
NCORES = 8
SHARD = N // NCORES          # 12500
BLK = 128                    # src rows per block
NBLK = (SHARD + BLK - 1) // BLK   # 98
TCOL = D_LABEL + IN_F + 1    # 289: [label | h | 1]

F32 = mybir.dt.float32
I32 = mybir.dt.int32


def _host_prep(h, label, W, adj_indices):
    src = np.asarray(adj_indices[0], dtype=np.int64)
    dst = np.asarray(adj_indices[1], dtype=np.int64)

    # concat table [N, 289] = [label | h | ones]
    table = np.empty((N, TCOL), dtype=np.float32)
    table[:, :D_LABEL] = label
    table[:, D_LABEL : D_LABEL + IN_F] = h
    table[:, TCOL - 1] = 1.0

    # sort edges by src once; then split per core / block
    order = np.argsort(src, kind="stable")
    s_sorted = src[order]
    d_sorted = dst[order]

    core_of = s_sorted // SHARD
    blk_of = (s_sorted % SHARD) // BLK

    # counts[m, b]
    counts = np.zeros((NCORES, NBLK), dtype=np.int64)
    np.add.at(counts, (core_of, blk_of), 1)
    tiles_per_blk = np.maximum((counts + 127) // 128, 1)  # [M, B]
    T_b = tiles_per_blk.max(axis=0)                       # shared schedule
    T_total = int(T_b.sum())

    # slot arrays per core, [T_total*128] padded with sentinels
    blk_tile_start = np.concatenate([[0], np.cumsum(T_b)])[:-1]  # tile idx of blk b

    dst_all = np.zeros((NCORES, T_total * 128), dtype=np.int32)
    srci_all = np.zeros((NCORES, T_total * 128), dtype=np.int32)
    srcl_all = np.full((NCORES, T_total * 128), 300.0, dtype=np.float32)

    # boundaries of each (core, blk) run inside the sorted edge list
    core_starts = np.searchsorted(s_sorted, np.arange(0, N + 1, SHARD))
    for m in range(NCORES):
        lo, hi = core_starts[m], core_starts[m + 1]
        s_m = s_sorted[lo:hi]
        d_m = d_sorted[lo:hi]
        b_m = (s_m % SHARD) // BLK
        # edges already sorted by src -> grouped by blk
        blk_bounds = np.searchsorted(b_m, np.arange(NBLK + 1))
        for b in range(NBLK):
            e0, e1 = blk_bounds[b], blk_bounds[b + 1]
            n_e = e1 - e0
            slot0 = blk_tile_start[b] * 128
            dst_all[m, slot0 : slot0 + n_e] = d_m[e0:e1]
            srci_all[m, slot0 : slot0 + n_e] = s_m[e0:e1] - m * SHARD
            srcl_all[m, slot0 : slot0 + n_e] = (s_m[e0:e1] - m * SHARD - b * BLK).astype(
                np.float32
            )

    # reshape to [128, T_total]: element [p, t] = edge slot p of tile t
    def to_pt(a):
        return np.ascontiguousarray(a.reshape(T_total, 128).T)

    # packed W: [128, 512] = [W[0:128,:] | W[128:256,:]]
    Wt = np.ascontiguousarray(
        np.concatenate([W[:128, :], W[128:, :]], axis=1), dtype=np.float32
    )

    iota_f = np.tile(np.arange(128, dtype=np.float32), (128, 1))
    ident = np.eye(128, dtype=np.float32)

    in_maps = []
    for m in range(NCORES):
        in_maps.append(
            {
                "table": table,
                "lsrc": np.ascontiguousarray(
                    label[m * SHARD : (m + 1) * SHARD], dtype=np.float32
                ),
                "dsti": to_pt(dst_all[m]),
                "srci": to_pt(srci_all[m]),
                "srcl": to_pt(srcl_all[m]),
                "wt": Wt,
                "iotaf": iota_f,
                "identf": ident,
            }
        )
    return in_maps, T_b, T_total


def _build_kernel(T_b, T_total):
    nc = bass.Bass()

    table = nc.dram_tensor("table", [N, TCOL], F32, kind="ExternalInput")
    lsrc_d = nc.dram_tensor("lsrc", [SHARD, D_LABEL], F32, kind="ExternalInput")
    dsti_d = nc.dram_tensor("dsti", [128, T_total], I32, kind="ExternalInput")
    srci_d = nc.dram_tensor("srci", [128, T_total], I32, kind="ExternalInput")
    srcl_d = nc.dram_tensor("srcl", [128, T_total], F32, kind="ExternalInput")
    wt_d = nc.dram_tensor("wt", [128, 2 * OUT_F], F32, kind="ExternalInput")
    iota_d = nc.dram_tensor("iotaf", [128, 128], F32, kind="ExternalInput")
    ident_d = nc.dram_tensor("identf", [128, 128], F32, kind="ExternalInput")
    out_d = nc.dram_tensor("out", [SHARD, OUT_F], F32, kind="ExternalOutput")

    with TileContext(nc) as tc:
        with (
            tc.tile_pool(name="const", bufs=1) as cpool,
            tc.tile_pool(name="gath", bufs=6) as gpool,
            tc.tile_pool(name="lsr", bufs=6) as lpool,
            tc.tile_pool(name="small", bufs=6) as spool,
            tc.tile_pool(name="sel", bufs=6) as selpool,
            tc.tile_pool(name="post", bufs=3) as postpool,
            tc.tile_pool(name="psA", bufs=2, space="PSUM") as psA,
            tc.tile_pool(name="psT", bufs=2, space="PSUM") as psT,
            tc.tile_pool(name="psO", bufs=2, space="PSUM") as psO,
        ):
            # constants (host-provided; on-device iota/affine_select miscompile)
            iota_f = cpool.tile([128, 128], F32, tag="iota_f")
            nc.sync.dma_start(out=iota_f[:], in_=iota_d[:, :])
            ident = cpool.tile([128, 128], F32, tag="ident")
            nc.sync.dma_start(out=ident[:], in_=ident_d[:, :])
            wt_sb = cpool.tile([128, 2 * OUT_F], F32, tag="wt")
            nc.sync.dma_start(out=wt_sb[:], in_=wt_d[:, :])
            dsti_sb = cpool.tile([128, T_total], I32, tag="dsti")
            nc.sync.dma_start(out=dsti_sb[:], in_=dsti_d[:, :])
            srci_sb = cpool.tile([128, T_total], I32, tag="srci")
            nc.sync.dma_start(out=srci_sb[:], in_=srci_d[:, :])
            srcl_sb = cpool.tile([128, T_total], F32, tag="srcl")
            nc.sync.dma_start(out=srcl_sb[:], in_=srcl_d[:, :])

            t = 0
            for b in range(NBLK):
                ntile = int(T_b[b])
                agg = psA.tile([128, IN_F + 1], F32, tag="agg")
                for j in range(ntile):
                    gath = gpool.tile([128, TCOL], F32, tag="gath")
                    nc.gpsimd.indirect_dma_start(
                        out=gath[:],
                        out_offset=None,
                        in_=table[:, :],
                        in_offset=IndirectOffsetOnAxis(ap=dsti_sb[:, t : t + 1], axis=0),
                    )
                    lsr = lpool.tile([128, D_LABEL], F32, tag="lsr")
                    nc.gpsimd.indirect_dma_start(
                        out=lsr[:],
                        out_offset=None,
                        in_=lsrc_d[:, :],
                        in_offset=IndirectOffsetOnAxis(ap=srci_sb[:, t : t + 1], axis=0),
                    )
                    prod = spool.tile([128, D_LABEL], F32, tag="prod")
                    dots = spool.tile([128, 1], F32, tag="dots")
                    nc.vector.tensor_tensor(
                        out=prod[:],
                        in0=lsr[:],
                        in1=gath[:, :D_LABEL],
                        op=mybir.AluOpType.mult,
                    )
                    nc.vector.tensor_reduce(
                        out=dots[:],
                        in_=prod[:],
                        axis=mybir.AxisListType.X,
                        op=mybir.AluOpType.add,
                    )
                    lr = spool.tile([128, 1], F32, tag="lr")
                    nc.vector.tensor_scalar(
                        out=lr[:],
                        in0=dots[:],
                        scalar1=ALPHA,
                        scalar2=dots[:],
                        op0=mybir.AluOpType.mult,
                        op1=mybir.AluOpType.max,
                    )
                    expv = spool.tile([128, 1], F32, tag="expv")
                    nc.scalar.activation(
                        expv[:], lr[:], mybir.ActivationFunctionType.Exp
                    )
                    S = selpool.tile([128, 128], F32, tag="S")
                    nc.vector.tensor_scalar(
                        out=S[:],
                        in0=iota_f[:],
                        scalar1=srcl_sb[:, t : t + 1],
                        scalar2=expv[:],
                        op0=mybir.AluOpType.is_equal,
                        op1=mybir.AluOpType.mult,
                    )
                    nc.tensor.matmul(
                        out=agg[:],
                        lhsT=S[:],
                        rhs=gath[:, D_LABEL:TCOL],
                        start=(j == 0),
                        stop=(j == ntile - 1),
                    )
                    t += 1

                # normalize + project
                rows = min(BLK, SHARD - b * BLK)
                rsm = spool.tile([128, 1], F32, tag="rsm")
                nc.vector.tensor_scalar_max(rsm[:], agg[:, IN_F : IN_F + 1], EPS)
                rcp = spool.tile([128, 1], F32, tag="rcp")
                nc.vector.reciprocal(rcp[:], rsm[:])
                scaled = postpool.tile([128, IN_F], F32, tag="scaled")
                nc.vector.tensor_scalar_mul(scaled[:], agg[:, :IN_F], rcp[:])
                outp = psO.tile([128, OUT_F], F32, tag="outp")
                for c in range(2):
                    tp = psT.tile([128, 128], F32, tag="tp")
                    nc.tensor.transpose(
                        out=tp[:], in_=scaled[:, c * 128 : (c + 1) * 128], identity=ident[:]
                    )
                    sT = postpool.tile([128, 128], F32, tag="sT")
                    nc.vector.tensor_copy(sT[:], tp[:])
                    nc.tensor.matmul(
                        out=outp[:],
                        lhsT=sT[:],
                        rhs=wt_sb[:, c * OUT_F : (c + 1) * OUT_F],
                        start=(c == 0),
                        stop=(c == 1),
                    )
                osb = postpool.tile([128, OUT_F], F32, tag="osb")
                nc.vector.tensor_copy(osb[:], outp[:])
                nc.sync.dma_start(
                    out=out_d[b * BLK : b * BLK + rows, :], in_=osb[:rows, :]
                )
    return nc


_CACHE = {}


def kernel(h, label, W, adj_indices):
    h = np.asarray(h, dtype=np.float32)
    label = np.asarray(label, dtype=np.float32)
    W = np.asarray(W, dtype=np.float32)
    adj_indices = np.asarray(adj_indices)

    in_maps, T_b, T_total = _host_prep(h, label, W, adj_indices)

    key = T_total
    if key not in _CACHE:
        _CACHE[key] = _build_kernel(T_b, T_total)
    nc = _CACHE[key]

    res = run_bass_kernel_spmd(nc, in_maps, core_ids=list(range(NCORES)))
    out = np.concatenate([r["out"] for r in res.results], axis=0)
    return out.astype(np.float32)

